# revision 1
# baseline (speedup 1.0000x reference)
# Trainium2 Bass kernel for nn_LongformerSelfAttentionPegasus (B=2,S=4096,D=768,
# H=12,HD=64, window W=256 one-sided, G=128 global prefix tokens).
#
# Sharding (8 NeuronCores): sequence-parallel — core c handles batch c//4,
# query rows [1024*(c%4), 1024*(c%4+1)). Banded attention is fully local (the
# host ships a +/-W halo of the hidden states). The global-query attention
# (rows 0..G attend to all S tokens through the *_global projections) is
# token-parallel: each core computes exp-score partials (numerator+denominator
# via a ones-column on V) over its own 1024 tokens, and a tiny [65*12, 128]
# AllReduce within each 4-core batch group completes the softmax. The final
# Dense + residual + LayerNorm are row-local, so no further communication.
#
# All matmul operands are bf16 (fp32 PSUM accumulation); fp32 matmuls would run
# at 1/4 PE rate. Softmax skips the max-subtraction (scores are provably tiny
# for this module: |s| < ~3), uses exp on ScalarE, and gets the denominator for
# free from the ones-column. Band masking (|rel|<=W, kpos>=G, kpos<S) is
# applied as a -30*maskbit PSUM-accumulated matmul against a constant -30*I.
import sys
import os as _os

for _p in ("/opt/trn_rl_repo",):
    if _p not in sys.path:
        sys.path.insert(0, _p)

import numpy as np
import ml_dtypes

import concourse.bass as bass
import concourse.bacc as bacc
import concourse.mybir as mybir
import concourse.tile as tile
from concourse import bass_utils

F32 = mybir.dt.float32
BF16 = mybir.dt.bfloat16
AF = mybir.ActivationFunctionType
ALU = mybir.AluOpType

B, S, D, H, HD = 2, 4096, 768, 12, 64
W, G = 256, 128
EPS = 1e-5
SCALE = 1.0 / np.sqrt(HD)

T = 1024                 # query rows per core
HALO = T + 2 * W         # 1536 banded kv rows per core
KT = G + HALO            # 1664 total kv rows (128 global + halo)
NCH = 8                  # query chunks of 128
NBT = HALO // 128        # 12 band kcol tiles
NKP = KT // 128          # 13 v partition tiles
MASK_NEG = -30.0
NEG_SLOPE = MASK_NEG     # value on the -30*I diagonal

# chunks covered by band tile t: max(0,t-4)..min(7,t)
def _tile_chunks(t):
    return max(0, t - 4), min(NCH - 1, t)

_NT = [(_tile_chunks(t)[1] - _tile_chunks(t)[0] + 1) for t in range(NBT)]
_MOFF = np.concatenate([[0], np.cumsum(np.array(_NT) * 128)]).astype(int)
MASK_COLS = int(_MOFF[-1])  # 5120


def _emit(tc, dt):
    nc = tc.nc
    xT_d, w_d, res_d, maskb_d, diag_d, msel_d, y_d = (
        dt["xT"], dt["w"], dt["res"], dt["maskb"], dt["diag"],
        dt["msel"], dt["y"])
    vrep_d = dt["vrep"]

    with (
        tc.tile_pool(name="const", bufs=1) as constp,
        tc.tile_pool(name="persist", bufs=1) as pers,
        tc.tile_pool(name="psc", bufs=2, space="PSUM") as psc,
        tc.tile_pool(name="pnum", bufs=2, space="PSUM") as pnum,
    ):
        # ---- ACT table warmup: attach table loads to dependency-free ops
        warm = constp.tile([1, 16], F32, tag="warm", name="warm")
        nc.vector.memset(warm[:], 1.0)
        nc.scalar.activation(warm[:], warm[:], AF.Exp)
        nc.scalar.activation(warm[:], warm[:], AF.Ln)
        nc.scalar.activation(warm[:], warm[:], AF.Square)
        nc.scalar.activation(warm[:], warm[:], AF.Identity, bias=warm[:, 0:1])

        # ---- constants ----
        diag = constp.tile([128, 128], BF16, tag="diag", name="diag")
        nc.sync.dma_start(diag[:], diag_d[:])
        maskb = constp.tile([128, MASK_COLS], BF16, tag="maskb", name="maskb")
        nc.sync.dma_start(maskb[:], maskb_d[:])
        msel = constp.tile([128, 2], F32, tag="msel", name="msel")
        nc.sync.dma_start(msel[:], msel_d[:])
        # per-partition bias tiles, one consolidated DMA: [128, 24]
        biasT = constp.tile([128, 24], F32, tag="biasT", name="biasT")
        nc.sync.dma_start(biasT[:], dt["biasT"][:])
        bias_t = {}
        for row, name in ((0, "bq"), (1, "bk"), (2, "bkg"), (3, "bqg")):
            bias_t[name] = [biasT[:, row * 6 + p:row * 6 + p + 1]
                            for p in range(6)]
        # free-dim vectors, host-replicated across partitions
        vrow = {}
        for row, name in ((0, "bv"), (1, "bvg"), (2, "gam"), (3, "bet")):
            t_ = constp.tile([128, D], F32, tag=name)
            nc.sync.dma_start(t_[:], vrep_d[row])
            vrow[name] = t_

        # ---- persistent activation storage (bf16) ----
        kT = [pers.tile([128, KT], BF16, tag=f"kT{p}", name=f"kT{p}") for p in range(6)]
        qT = [pers.tile([128, T], BF16, tag=f"qT{p}", name=f"qT{p}") for p in range(6)]
        kgfT = [pers.tile([128, T], BF16, tag=f"kgfT{p}", name=f"kgfT{p}") for p in range(6)]
        qgT = [pers.tile([128, G], BF16, tag=f"qgT{p}", name=f"qgT{p}") for p in range(6)]
        vsb = [pers.tile([128, H * (HD + 1)], BF16, tag=f"v{p}", name=f"v{p}") for p in range(NKP)]
        vgf = [pers.tile([128, H * (HD + 1)], BF16, tag=f"vg{p}", name=f"vg{p}") for p in range(8)]
        ctxT = [pers.tile([128, T], BF16, tag=f"ctxT{p}", name=f"ctxT{p}") for p in range(6)]
        ogsb = pers.tile([128, H * G], F32, tag="ogsb", name="ogsb")
        ogred = pers.tile([128, H * G], F32, tag="ogred", name="ogred")

        nc.gpsimd.memset(ogsb[:], 0.0)
        # ones columns of the v tiles (ones-trick denominator)
        for p in range(NKP):
            nc.gpsimd.memset(vsb[p][:], 1.0)
        for p in range(8):
            nc.gpsimd.memset(vgf[p][:], 1.0)

        # ---- projections ----
        with tc.tile_pool(name="xw", bufs=2) as xw:
            xT = [pers.tile([128, KT], BF16, tag=f"xT{p}", name=f"xT{p}") for p in range(6)]
            for p in range(6):
                nc.sync.dma_start(xT[p][:], xT_d[128 * p:128 * p + 128, :])

            def load_w(widx):
                tiles = []
                for k in range(6):
                    t_ = xw.tile([128, D], BF16, tag=f"w{k}", name=f"w{k}")
                    nc.sync.dma_start(t_[:], w_d[widx, 128 * k:128 * k + 128, :])
                    tiles.append(t_)
                return tiles

            def proj_T(wt, out_tiles, bias, xcol0, ncols):
                # out[dout, tok] = W.T @ x.T  (lhsT=W tile, rhs=xT slice)
                for m in range(6):
                    nn = 0
                    while nn < ncols:
                        nw = min(512, ncols - nn)
                        ps = psc.tile([128, 640], F32, tag="sc", name="sc")
                        for k in range(6):
                            nc.tensor.matmul(
                                ps[:, :nw],
                                wt[k][:, 128 * m:128 * m + 128],
                                xT[k][:, xcol0 + nn:xcol0 + nn + nw],
                                start=(k == 0), stop=(k == 5))
                        nc.scalar.activation(out_tiles[m][:, nn:nn + nw],
                                             ps[:, :nw], AF.Identity,
                                             bias=bias[m][:], scale=1.0)
                        nn += nw

            def proj_N(wt, out_tiles, brow, xcol0, ntok):
                # out[tok, dout] natural layout, strided (HD+1) per head.
                for m in range(ntok // 128):
                    for n0, nw in ((0, 512), (512, 256)):
                        ps = psc.tile([128, 640], F32, tag="sc", name="sc")
                        for k in range(6):
                            nc.tensor.matmul(
                                ps[:, :nw],
                                xT[k][:, xcol0 + 128 * m:xcol0 + 128 * m + 128],
                                wt[k][:, n0:n0 + nw],
                                start=(k == 0), stop=(k == 5))
                        ov = out_tiles[m][:].rearrange("p (h e) -> p h e", e=HD + 1)
                        h0 = n0 // HD
                        nh = nw // HD
                        nc.vector.tensor_tensor(
                            ov[:, h0:h0 + nh, :HD],
                            ps[:, :nw].rearrange("p (h e) -> p h e", e=HD),
                            vrow[brow][:, n0:n0 + nw]
                            .rearrange("p (h e) -> p h e", e=HD),
                            ALU.add)

            wk = load_w(1)
            proj_T(wk, kT, bias_t["bk"], 0, KT)
            wv = load_w(2)
            proj_N(wv, vsb, "bv", 0, KT)
            wq = load_w(0)
            proj_T(wq, qT, bias_t["bq"], G + W, T)
            wkg = load_w(3)
            proj_T(wkg, kgfT, bias_t["bkg"], G + W, T)
            wvg = load_w(4)
            proj_N(wvg, vgf, "bvg", G + W, T)
            wqg = load_w(5)
            proj_T(wqg, qgT, bias_t["bqg"], 0, G)

        if _os.environ.get("OG_OFF") != "1":
            # ---- global-query attention partials (then kick the AllReduce) ----
            with tc.tile_pool(name="ptog", bufs=16) as ptogp:
                ptog = {}
                for pr in range(6):
                    for tt in range(8):
                        pt = ptogp.tile([128, 256], BF16, tag="ptog", name="ptog")
                        for half in (0, 1):
                            r0, r1 = 64 * half, 64 * half + 64
                            ps = psc.tile([128, 640], F32, tag="sc", name="sc")
                            nc.tensor.matmul(
                                ps[:, :128],
                                kgfT[pr][r0:r1, 128 * tt:128 * tt + 128],
                                qgT[pr][r0:r1, :],
                                start=True, stop=True)
                            nc.scalar.activation(
                                pt[:, 128 * half:128 * half + 128],
                                ps[:, :128], AF.Exp)
                        ptog[(pr, tt)] = pt
                for h in range(H):
                    pr, half = h // 2, h % 2
                    ps = pnum.tile([65, 1024], F32, tag="num", name="num")
                    for tt in range(8):
                        nc.tensor.matmul(
                            ps[:, :G],
                            vgf[tt][:, (HD + 1) * h:(HD + 1) * h + HD + 1],
                            ptog[(pr, tt)][:, 128 * half:128 * half + 128],
                            start=(tt == 0), stop=(tt == 7))
                    nc.scalar.copy(ogsb[:65, G * h:G * h + G], ps[:, :G])
                with tc.tile_pool(name="ogdram", bufs=1, space="DRAM") as ogd:
                    og_in = ogd.tile([128, H * G], F32, tag="og_in", name="og_in")
                    og_out = ogd.tile([128, H * G], F32, tag="og_out",
                                      name="og_out")
                    nc.sync.dma_start(og_in[:], ogsb[:])
                    if _os.environ.get("NO_CC") == "1":
                        nc.sync.dma_start(og_out[:], og_in[:])
                    else:
                        nc.gpsimd.collective_compute(
                            "AllReduce", ALU.add,
                            replica_groups=[[0, 1, 2, 3], [4, 5, 6, 7]],
                            ins=[og_in.opt()], outs=[og_out.opt()])
                    nc.sync.dma_start(ogred[:], og_out[:])

        # ---- banded local attention ----
        with (
            tc.tile_pool(name="pt", bufs=13) as ptp,
            tc.tile_pool(name="ptg", bufs=2) as ptgp,
            tc.tile_pool(name="dsc", bufs=3) as dscp,
        ):
            for h in range(H):
                pr, half = h // 2, h % 2
                r0, r1 = 64 * half, 64 * half + 64
                # global columns: [128 gcols, 1024 qrows]
                ptg = ptgp.tile([128, T], BF16, tag="ptg", name="ptg")
                for n0 in (0, 512):
                    ps = psc.tile([128, 640], F32, tag="sc", name="sc")
                    nc.tensor.matmul(ps[:, :512],
                                     kT[pr][r0:r1, 0:G],
                                     qT[pr][r0:r1, n0:n0 + 512],
                                     start=True, stop=True)
                    nc.scalar.activation(ptg[:, n0:n0 + 512], ps[:, :512], AF.Exp)
                pts = []
                # band score rects: [128 kcols, n_t*128 qrows], exp -> bf16
                for t in range(NBT):
                    lo, hi = _tile_chunks(t)
                    nq = (hi - lo + 1) * 128
                    ps = psc.tile([128, 640], F32, tag="sc", name="sc")
                    n0 = 0
                    while n0 < nq:
                        nw = min(512, nq - n0)
                        nc.tensor.matmul(
                            ps[:, n0:n0 + nw],
                            kT[pr][r0:r1, G + 128 * t:G + 128 * t + 128],
                            qT[pr][r0:r1, 128 * lo + n0:128 * lo + n0 + nw],
                            start=True, stop=False)
                        nc.tensor.matmul(
                            ps[:, n0:n0 + nw],
                            diag[:],
                            maskb[:, _MOFF[t] + n0:_MOFF[t] + n0 + nw],
                            start=False, stop=True)
                        n0 += nw
                    pt = ptp.tile([128, 640], BF16, tag="pt", name="pt")
                    nc.scalar.activation(pt[:, :nq], ps[:, :nq], AF.Exp)
                    pts.append(pt)
                # PV with ones column; chunks write disjoint 128-col groups
                ps = pnum.tile([65, 1024], F32, tag="num", name="num")
                vcol = slice((HD + 1) * h, (HD + 1) * h + HD + 1)
                for q in range(NCH):
                    first_of_bank = q in (0, 4)
                    for i, t in enumerate(range(q, q + 5)):
                        lo, _ = _tile_chunks(t)
                        nc.tensor.matmul(
                            ps[:, 128 * q:128 * q + 128],
                            vsb[1 + t][:, vcol],
                            pts[t][:, 128 * (q - lo):128 * (q - lo) + 128],
                            start=(first_of_bank and i == 0), stop=False,
                            skip_group_check=not (first_of_bank and i == 0))
                    nc.tensor.matmul(
                        ps[:, 128 * q:128 * q + 128],
                        vsb[0][:, vcol],
                        ptg[:, 128 * q:128 * q + 128],
                        start=False, stop=(q in (3, 7)),
                        skip_group_check=q not in (3, 7))
                # softmax denominator -> reciprocal via exp(-ln(x)), in place
                inv = dscp.tile([1, T], F32, tag="inv", name="inv", bufs=2)
                nc.scalar.activation(inv[:], ps[64:65, :], AF.Ln)
                nc.scalar.activation(inv[:], inv[:], AF.Exp, scale=-1.0)
                invb = dscp.tile([HD, T], F32, tag="invb", name="invb", bufs=2)
                nc.gpsimd.partition_broadcast(invb[:], inv[:])
                nc.vector.tensor_tensor(
                    ctxT[pr][r0:r1, :], ps[:HD, :], invb[:], ALU.mult)

            if _os.environ.get("OG_OFF") != "1":
                # ---- fold the AllReduced global-attention output into ctxT ----
                inv = dscp.tile([1, H * G], F32, tag="oginv", name="oginv", bufs=1)
                nc.scalar.activation(inv[:], ogred[64:65, :], AF.Ln)
                nc.scalar.activation(inv[:], inv[:], AF.Exp, scale=-1.0)
                invb = dscp.tile([HD, H * G], F32, tag="oginvb", name="oginvb", bufs=1)
                nc.gpsimd.partition_broadcast(invb[:], inv[:])
                for h in range(H):
                    pr, half = h // 2, h % 2
                    r0, r1 = 64 * half, 64 * half + 64
                    ogt = dscp.tile([128, G], BF16, tag="ogt", name="ogt", bufs=2)
                    nc.vector.tensor_tensor(
                        ogt[r0:r1, :], ogred[:HD, G * h:G * h + G],
                        invb[:, G * h:G * h + G], ALU.mult)
                    nc.vector.tensor_scalar_mul(
                        ctxT[pr][r0:r1, :G], ctxT[pr][r0:r1, :G],
                        msel[r0:r1, 1:2])
                    nc.vector.scalar_tensor_tensor(
                        ctxT[pr][r0:r1, :G], ogt[r0:r1, :], msel[r0:r1, 0:1],
                        ctxT[pr][r0:r1, :G], ALU.mult, ALU.add)

        # ---- output Dense + residual + LayerNorm ----
        with (
            tc.tile_pool(name="wo", bufs=1) as wop,
            tc.tile_pool(name="ln", bufs=3) as lnp,
        ):
            wo = []
            for k in range(6):
                t_ = wop.tile([128, D], BF16, tag=f"wo{k}", name=f"wo{k}")
                nc.sync.dma_start(t_[:], w_d[6, 128 * k:128 * k + 128, :])
                wo.append(t_)
            epst = wop.tile([128, 1], F32, tag="epst", name="epst")
            nc.gpsimd.memset(epst[:], EPS)
            for m in list(range(1, 8)) + [0]:
                ys = lnp.tile([128, D], F32, tag="ys", name="ys")
                rs = lnp.tile([128, D], F32, tag="rs", name="rs")
                nc.sync.dma_start(rs[:], res_d[128 * m:128 * m + 128, :])
                sums = lnp.tile([128, 4], F32, tag="sums", name="sums")
                for n0, nw in ((0, 512), (512, 256)):
                    ps = psc.tile([128, 640], F32, tag="sc", name="sc")
                    for k in range(6):
                        nc.tensor.matmul(
                            ps[:, :nw],
                            ctxT[k][:, 128 * m:128 * m + 128],
                            wo[k][:, n0:n0 + nw],
                            start=(k == 0), stop=(k == 5))
                    nc.vector.scalar_tensor_tensor(
                        ys[:, n0:n0 + nw], ps[:, :nw], 1.0, rs[:, n0:n0 + nw],
                        ALU.mult, ALU.add,
                        accum_out=sums[:, (0 if n0 == 0 else 1):(1 if n0 == 0 else 2)])
                nc.vector.tensor_add(sums[:, 2:3], sums[:, 0:1], sums[:, 1:2])
                negmean = lnp.tile([128, 1], F32, tag="negmean", name="negmean")
                nc.vector.tensor_scalar_mul(negmean[:], sums[:, 2:3], -1.0 / D)
                yc = lnp.tile([128, D], F32, tag="yc", name="yc")
                nc.scalar.activation(yc[:], ys[:], AF.Identity,
                                     bias=negmean[:], scale=1.0)
                sq = lnp.tile([128, D], F32, tag="sq", name="sq")
                sumsq = lnp.tile([128, 1], F32, tag="sumsq", name="sumsq")
                nc.scalar.activation(sq[:], yc[:], AF.Square,
                                     accum_out=sumsq[:])
                lnv = lnp.tile([128, 1], F32, tag="lnv", name="lnv")
                nc.scalar.activation(lnv[:], sumsq[:], AF.Ln,
                                     bias=epst[:], scale=1.0 / D)
                istd = lnp.tile([128, 1], F32, tag="istd", name="istd")
                nc.scalar.activation(istd[:], lnv[:], AF.Exp, scale=-0.5)
                yo = lnp.tile([128, D], F32, tag="yo", name="yo")
                nc.vector.scalar_tensor_tensor(
                    yo[:], yc[:], istd[:], vrow["gam"][:], ALU.mult, ALU.mult)
                nc.vector.tensor_tensor(
                    yo[:], yo[:], vrow["bet"][:], ALU.add)
                nc.sync.dma_start(y_d[128 * m:128 * m + 128, :], yo[:])


def build_nc():
    nc = bacc.Bacc(trn_type="TRN2", num_devices=8)
    dt = {
        "xT": nc.dram_tensor("xT", [D, KT], BF16, kind="ExternalInput"),
        "w": nc.dram_tensor("w", [7, D, D], BF16, kind="ExternalInput"),
        "res": nc.dram_tensor("res", [T, D], F32, kind="ExternalInput"),
        "maskb": nc.dram_tensor("maskb", [128, MASK_COLS], BF16,
                                kind="ExternalInput"),
        "diag": nc.dram_tensor("diag", [128, 128], BF16, kind="ExternalInput"),
        "msel": nc.dram_tensor("msel", [128, 2], F32, kind="ExternalInput"),
        "vrep": nc.dram_tensor("vrep", [4, 128, D], F32, kind="ExternalInput"),
        "biasT": nc.dram_tensor("biasT", [128, 24], F32, kind="ExternalInput"),
        "y": nc.dram_tensor("y", [T, D], F32, kind="ExternalOutput"),
    }
    with tile.TileContext(nc) as tc:
        _emit(tc, dt)
    nc.compile()
    return nc


def host_inputs(inputs):
    """Build the 8 per-core input maps from the full problem inputs."""
    hs = np.asarray(inputs["hidden_states"], np.float32)
    assert hs.shape == (B, S, D)
    bf = lambda a: np.ascontiguousarray(np.asarray(a, np.float32)).astype(
        ml_dtypes.bfloat16)
    f32 = lambda a: np.ascontiguousarray(np.asarray(a, np.float32))

    wstack = np.stack([
        np.asarray(inputs["Wq"], np.float32) * SCALE,
        np.asarray(inputs["Wk"], np.float32),
        np.asarray(inputs["Wv"], np.float32),
        np.asarray(inputs["Wkg"], np.float32),
        np.asarray(inputs["Wvg"], np.float32),
        np.asarray(inputs["Wqg"], np.float32) * SCALE,
        np.asarray(inputs["Wo"], np.float32),
    ])
    vecs = np.stack([
        np.asarray(inputs["bq"], np.float32) * SCALE,
        np.asarray(inputs["bk"], np.float32),
        np.asarray(inputs["bkg"], np.float32),
        np.asarray(inputs["bqg"], np.float32) * SCALE,
        np.asarray(inputs["bv"], np.float32),
        np.asarray(inputs["bvg"], np.float32),
        np.asarray(inputs["ln_gamma"], np.float32),
        np.asarray(inputs["ln_beta"], np.float32),
    ])
    bo = np.asarray(inputs["bo"], np.float32)
    biasT = np.zeros((128, 24), np.float32)
    for row in range(4):
        for p in range(6):
            biasT[:, row * 6 + p] = vecs[row, 128 * p:128 * p + 128]
    vrep = np.ascontiguousarray(np.broadcast_to(
        np.stack([
            np.asarray(inputs["bv"], np.float32),
            np.asarray(inputs["bvg"], np.float32),
            np.asarray(inputs["ln_gamma"], np.float32),
            np.asarray(inputs["ln_beta"], np.float32),
        ])[:, None, :], (4, 128, D)))
    diag = (MASK_NEG * np.eye(128, dtype=np.float32))

    w_bf = bf(wstack)
    vecs_f = f32(vecs)
    diag_bf = bf(diag)

    in_maps = []
    for c in range(8):
        b, j = c // 4, c % 4
        r0 = j * T
        x = hs[b]
        xp = np.zeros((S + 2 * W, D), np.float32)
        xp[W:W + S] = x
        x_kv = np.concatenate([x[:G], xp[r0:r0 + HALO]], axis=0)  # [1664, D]
        xT = bf(x_kv.T)
        res = f32(x[r0:r0 + T] + bo)

        mask = np.zeros((128, MASK_COLS), np.float32)
        for t in range(NBT):
            lo, hi = _tile_chunks(t)
            nq = (hi - lo + 1) * 128
            jj = np.arange(128 * t, 128 * t + 128)[:, None]
            ii = np.arange(lo * 128, lo * 128 + nq)[None, :]
            kpos = r0 - W + jj
            valid = ((jj - ii >= 0) & (jj - ii <= 2 * W)
                     & (kpos >= G) & (kpos < S))
            mask[:, _MOFF[t]:_MOFF[t] + nq] = (~valid).astype(np.float32)

        m = 1.0 if j == 0 else 0.0
        msel = np.zeros((128, 2), np.float32)
        msel[:, 0] = m
        msel[:, 1] = 1.0 - m

        in_maps.append({
            "xT": xT, "w": w_bf, "res": res,
            "maskb": bf(mask), "diag": diag_bf, "msel": f32(msel),
            "vrep": vrep, "biasT": biasT,
        })
    return in_maps


_NC_CACHE = {}


def _get_nc():
    if "nc" not in _NC_CACHE:
        _NC_CACHE["nc"] = build_nc()
    return _NC_CACHE["nc"]


def kernel(**inputs) -> np.ndarray:
    # sanity-check the fixed global-attention pattern this kernel hardcodes
    iga = np.asarray(inputs["is_index_global_attn"])
    assert iga.shape == (B, S)
    expect = np.broadcast_to(np.arange(S) < G, (B, S))
    assert np.array_equal(iga, expect), "kernel hardcodes a G=128 prefix"
    am = np.asarray(inputs["attention_mask"], np.float32)
    assert np.all(am == 0.0), "kernel assumes no key-padding mask"

    nc = _get_nc()
    in_maps = host_inputs(inputs)
    res = bass_utils.run_bass_kernel_spmd(nc, in_maps, core_ids=list(range(8)))
    outs = res.results if hasattr(res, "results") else res
    y = np.zeros((B, S, D), np.float32)
    for c in range(8):
        b, j = c // 4, c % 4
        y[b, j * T:(j + 1) * T] = outs[c]["y"]
    return y


if __name__ == "__main__":
    nc = build_nc()
    print("build ok; instructions:",
          sum(len(bb.instructions) for bb in nc.main_func.blocks))



# revision 14
# speedup vs baseline: 1.3479x; 1.3479x over previous
# Trainium2 Bass kernel for nn_LongformerSelfAttentionPegasus (B=2,S=4096,D=768,
# H=12,HD=64, window W=256 one-sided, G=128 global prefix tokens).
#
# Sharding (8 NeuronCores): sequence-parallel — core c handles batch c//4,
# query rows [1024*(c%4), 1024*(c%4+1)). Banded attention is fully local (the
# host ships a +/-W halo of the hidden states). The global-query attention
# (rows 0..G attend to all S tokens through the *_global projections) is
# token-parallel: each core computes exp-score partials (numerator+denominator
# via a ones-column on V) over its own 1024 tokens, and a [65*12, 128]
# AllReduce within each 4-core batch group completes the softmax. The final
# Dense + residual + LayerNorm are row-local, so no further communication.
#
# v2 restructure (from the v1 per-band-tile layout):
#  - scores are computed per (head, query-chunk) into a [128, 768] PSUM tile
#    (5 banded key blocks + 1 global-key block, 128 queries each) -> ONE exp
#    per chunk instead of one per band tile.
#  - band masking only where needed: outer-diagonal blocks are triangles, the
#    three inner diagonals are always valid; sequence-boundary masking is
#    folded into the same per-core mask data. Mask adds are issued as two
#    concurrent half-row matmuls (row groups 0-63 / 64-127 of the PE array).
#  - softmax reciprocals via vector.reciprocal_approx_fast (custom DVE op)
#    and the LayerNorm via Rsqrt: no Ln/Exp round trips -> no ACT table
#    thrash (v1 paid 43 table loads at 1.28us each).
#  - PV matmuls for chunk c are emitted after the score matmuls of chunk c+1
#    so the in-order PE queue never stalls on the exp latency.
#  - v / v_global biases are folded in after the softmax normalize
#    (out = raw/den + bias since probabilities sum to 1); k / k_global biases
#    are dropped entirely (softmax shift invariance).
import sys
import os as _os

for _p in ("/opt/trn_rl_repo",):
    if _p not in sys.path:
        sys.path.insert(0, _p)

import numpy as np
import ml_dtypes

import concourse.bass as bass
import concourse.bacc as bacc
import concourse.mybir as mybir
import concourse.tile as tile
from concourse import bass_utils

F32 = mybir.dt.float32
BF16 = mybir.dt.bfloat16
AF = mybir.ActivationFunctionType
ALU = mybir.AluOpType

B, S, D, H, HD = 2, 4096, 768, 12, 64
W, G = 256, 128
EPS = 1e-5
SCALE = 1.0 / np.sqrt(HD)

T = 1024                 # query rows per core
HALO = T + 2 * W         # 1536 banded kv rows per core
KT = G + HALO            # 1664 total kv rows (128 global + halo)
NCH = 8                  # query chunks of 128
NBT = HALO // 128        # 12 band kcol tiles
NKP = KT // 128          # 13 v partition tiles
MASK_NEG = -30.0

# sc-block order within a chunk: positions 0..4 hold band blocks
# i = 0,4,1,2,3 (i = key-tile offset, t = chunk+i); position 5 = global cols.
POS2I = (0, 4, 1, 2, 3)
# mask spans per chunk: (pos0, nblocks) runs of sc blocks that need a mask
# matmul on at least one core. i=0 / i=4 are band triangles everywhere;
# chunks 0/1 additionally hit the kpos<G boundary (tiles t<=2), chunk 7 the
# kpos>=S boundary (tiles t>=10, i.e. i=3 on top of i=0/4).
MASK_SPANS = {
    c: ([(0, 4)] if c == 0 else
        [(0, 3)] if c == 1 else
        [(0, 2), (4, 1)] if c == 7 else
        [(0, 2)])
    for c in range(NCH)
}
_off = 0
MB_OFF = {}
for _c in range(NCH):
    MB_OFF[_c] = []
    for (_p0, _nb) in MASK_SPANS[_c]:
        MB_OFF[_c].append(_off)
        _off += 128 * _nb
MASK_COLS = _off  # 2560


def _emit(tc, dt):
    nc = tc.nc
    xT_d, w_d, res_d, maskb_d, diag_d, msel_d, y_d = (
        dt["xT"], dt["w"], dt["res"], dt["maskb"], dt["diag"],
        dt["msel"], dt["y"])
    vrep_d = dt["vrep"]

    with (
        tc.tile_pool(name="const", bufs=1) as constp,
        tc.tile_pool(name="persist", bufs=1) as pers,
    ):
        # ---- ACT table warmup: exp set only (identity/square/copy share it)
        warm = constp.tile([1, 16], F32, tag="warm", name="warm")
        nc.vector.memset(warm[:], 1.0)
        nc.scalar.activation(warm[:], warm[:], AF.Exp)
        nc.scalar.activation(warm[:], warm[:], AF.Square)
        nc.scalar.activation(warm[:], warm[:], AF.Identity, bias=warm[:, 0:1])

        # ---- constants ----
        diag = constp.tile([128, 128], BF16, tag="diag", name="diag")
        nc.sync.dma_start(diag[:], diag_d[:])
        maskb = constp.tile([128, MASK_COLS], BF16, tag="maskb", name="maskb")
        nc.sync.dma_start(maskb[:], maskb_d[:])
        msel = constp.tile([128, 2], F32, tag="msel", name="msel")
        nc.sync.dma_start(msel[:], msel_d[:])
        # per-partition bias columns for q / qg projections: [128, 24]
        biasT = constp.tile([128, 24], F32, tag="biasT", name="biasT")
        nc.sync.dma_start(biasT[:], dt["biasT"][:])
        bias_t = {}
        for row, name in ((0, "bq"), (3, "bqg")):
            bias_t[name] = [biasT[:, row * 6 + p:row * 6 + p + 1]
                            for p in range(6)]
        # per-(partition,head) bias columns for the post-normalize v/vg folds
        bcols = constp.tile([128, 24], F32, tag="bcols", name="bcols")
        nc.sync.dma_start(bcols[:], dt["bcols"][:])
        # free-dim vectors (gamma/beta), host-replicated across partitions
        vrow = {}
        for row, name in ((0, "gam"), (1, "bet")):
            t_ = constp.tile([128, D], F32, tag=name)
            nc.sync.dma_start(t_[:], vrep_d[row])
            vrow[name] = t_

        # ---- persistent activation storage (bf16) ----
        kT = [pers.tile([128, KT], BF16, tag=f"kT{p}", name=f"kT{p}") for p in range(6)]
        qT = [pers.tile([128, T], BF16, tag=f"qT{p}", name=f"qT{p}") for p in range(6)]
        kgfT = [pers.tile([128, T], BF16, tag=f"kgfT{p}", name=f"kgfT{p}") for p in range(6)]
        qgT = [pers.tile([128, G], BF16, tag=f"qgT{p}", name=f"qgT{p}") for p in range(6)]
        vsb = [pers.tile([128, H * (HD + 1)], BF16, tag=f"v{p}", name=f"v{p}") for p in range(NKP)]
        vgf = [pers.tile([128, H * (HD + 1)], BF16, tag=f"vg{p}", name=f"vg{p}") for p in range(8)]
        ctxT = [pers.tile([128, T], BF16, tag=f"ctxT{p}", name=f"ctxT{p}") for p in range(6)]
        ogsb = pers.tile([65, H * G], F32, tag="ogsb", name="ogsb")
        ogred = pers.tile([65, H * G], F32, tag="ogred", name="ogred")


        # ones columns of the v tiles (ones-trick denominator), strided memset
        for p in range(NKP):
            ov = vsb[p][:].rearrange("p (h e) -> p h e", e=HD + 1)
            nc.vector.memset(ov[:, :, HD:HD + 1], 1.0)
        for p in range(8):
            ov = vgf[p][:].rearrange("p (h e) -> p h e", e=HD + 1)
            nc.vector.memset(ov[:, :, HD:HD + 1], 1.0)

        # ---- projections ----
        with (
            tc.tile_pool(name="xw", bufs=2) as xw,
            tc.tile_pool(name="psp", bufs=4, space="PSUM") as psp,
        ):
            xT = [xw.tile([128, KT], BF16, tag=f"xT{p}", name=f"xT{p}")
                  for p in range(6)]
            for p in range(6):
                nc.sync.dma_start(xT[p][:], xT_d[128 * p:128 * p + 128, :])

            def load_w(widx):
                tiles = []
                for k in range(6):
                    t_ = xw.tile([128, D], BF16, tag=f"w{k}", name=f"w{k}")
                    nc.sync.dma_start(t_[:], w_d[widx, 128 * k:128 * k + 128, :])
                    tiles.append(t_)
                return tiles

            def proj_T(wt, out_tiles, bias, xcol0, ncols):
                # out[dout, tok] = W.T @ x.T  (lhsT=W tile, rhs=xT slice)
                for m in range(6):
                    nn = 0
                    while nn < ncols:
                        nw = min(512, ncols - nn)
                        ps = psp.tile([128, 512], F32, tag="sc", name="sc")
                        for k in range(6):
                            nc.tensor.matmul(
                                ps[:, :nw],
                                wt[k][:, 128 * m:128 * m + 128],
                                xT[k][:, xcol0 + nn:xcol0 + nn + nw],
                                start=(k == 0), stop=(k == 5))
                        if bias is None:
                            nc.vector.tensor_copy(
                                out_tiles[m][:, nn:nn + nw], ps[:, :nw])
                        else:
                            nc.vector.tensor_scalar_add(
                                out_tiles[m][:, nn:nn + nw], ps[:, :nw],
                                bias[m][:])
                        nn += nw

            def proj_N(wt, out_tiles, xcol0, ntok):
                # out[tok, dout] natural layout, strided (HD+1) per head.
                for m in range(ntok // 128):
                    for n0, nw in ((0, 512), (512, 256)):
                        ps = psp.tile([128, 512], F32, tag="sc", name="sc")
                        for k in range(6):
                            nc.tensor.matmul(
                                ps[:, :nw],
                                xT[k][:, xcol0 + 128 * m:xcol0 + 128 * m + 128],
                                wt[k][:, n0:n0 + nw],
                                start=(k == 0), stop=(k == 5))
                        ov = out_tiles[m][:].rearrange("p (h e) -> p h e", e=HD + 1)
                        h0 = n0 // HD
                        nh = nw // HD
                        nc.vector.tensor_copy(
                            ov[:, h0:h0 + nh, :HD],
                            ps[:, :nw].rearrange("p (h e) -> p h e", e=HD))

            wk = load_w(1)
            proj_T(wk, kT, None, 0, KT)
            wv = load_w(2)
            proj_N(wv, vsb, 0, KT)
            wq = load_w(0)
            proj_T(wq, qT, bias_t["bq"], G + W, T)
            wkg = load_w(3)
            proj_T(wkg, kgfT, None, G + W, T)
            wvg = load_w(4)
            proj_N(wvg, vgf, G + W, T)
            wqg = load_w(5)
            proj_T(wqg, qgT, bias_t["bqg"], 0, G)

        if _os.environ.get("OG_OFF") != "1":
            # ---- global-query attention partials (then kick the AllReduce)
            with (
                tc.tile_pool(name="ptog", bufs=6) as ptogp,
                tc.tile_pool(name="pps", bufs=2, space="PSUM") as pps,
                tc.tile_pool(name="ogn", bufs=2, space="PSUM") as ognp,
            ):
                for pr in range(6):
                    # [128, 1024] psum per tt-quad: half 0 -> bank 0
                    # (cols 0:512), half 1 -> bank 1 (cols 512:1024) so the
                    # row-group-concurrent matmul pairs drain to different
                    # banks (same-bank cross-row-group writes hang the PE).
                    ptq = []
                    for q in range(2):
                        ps = pps.tile([128, 1024], F32, tag="pp", name="pp")
                        for k in range(4):
                            tt = 4 * q + k
                            for half in (0, 1):
                                r0 = 64 * half
                                nc.tensor.matmul(
                                    ps[:, 512 * half + 128 * k:
                                       512 * half + 128 * k + 128],
                                    kgfT[pr][r0:r0 + 64,
                                             128 * tt:128 * tt + 128],
                                    qgT[pr][r0:r0 + 64, :],
                                    start=True, stop=True)
                        pt = ptogp.tile([128, 1024], BF16, tag="ptog",
                                        name="ptog")
                        nc.scalar.activation(pt[:], ps[:], AF.Exp)
                        ptq.append(pt)
                    for half in (0, 1):
                        h = 2 * pr + half
                        vcol = slice((HD + 1) * h, (HD + 1) * h + HD + 1)
                        ps2 = ognp.tile([65, 128], F32, tag="ogn", name="ogn")
                        for tt in range(8):
                            rhs = ptq[tt // 4][
                                :, 512 * half + 128 * (tt % 4):
                                512 * half + 128 * (tt % 4) + 128]
                            nc.tensor.matmul(
                                ps2[:, :], vgf[tt][:, vcol], rhs,
                                start=(tt == 0), stop=(tt == 7))
                        nc.scalar.copy(ogsb[:, G * h:G * h + G], ps2[:])
                with tc.tile_pool(name="ogdram", bufs=1, space="DRAM") as ogd:
                    og_in = ogd.tile([65, H * G], F32, tag="og_in",
                                     name="og_in")
                    og_out = ogd.tile([65, H * G], F32, tag="og_out",
                                      name="og_out")
                    nc.sync.dma_start(og_in[:], ogsb[:])
                    if _os.environ.get("NO_CC") == "1":
                        nc.sync.dma_start(og_out[:], og_in[:])
                    else:
                        nc.gpsimd.collective_compute(
                            "AllReduce", ALU.add,
                            replica_groups=[[0, 1, 2, 3], [4, 5, 6, 7]],
                            ins=[og_in.opt()], outs=[og_out.opt()])
                    nc.sync.dma_start(ogred[:], og_out[:])

        # ---- banded local attention ----
        with (
            tc.tile_pool(name="scp", bufs=2, space="PSUM") as scp,
            tc.tile_pool(name="pnum", bufs=2, space="PSUM") as pnp,
            tc.tile_pool(name="ptsp", bufs=3) as ptsp,
            tc.tile_pool(name="invp", bufs=2) as invp,
            tc.tile_pool(name="denp", bufs=3) as denp,
            tc.tile_pool(name="dsc", bufs=2) as dscp,
        ):
            mask_pos = {c: set() for c in range(NCH)}
            for c in range(NCH):
                for (p0, nb) in MASK_SPANS[c]:
                    mask_pos[c].update(range(p0, p0 + nb))

            deferred = []

            def flush():
                while deferred:
                    deferred.pop(0)()

            def make_pv(h, c, pts, pnum):
                pr, half = h // 2, h % 2
                vcol = slice((HD + 1) * h, (HD + 1) * h + HD + 1)

                def emit():
                    first_of_bank = c in (0, 4)
                    for n, pos in enumerate(range(6)):
                        t = c + POS2I[pos] if pos < 5 else None
                        vt = vsb[1 + t] if pos < 5 else vsb[0]
                        first = first_of_bank and n == 0
                        last = (c in (3, 7)) and n == 5
                        nc.tensor.matmul(
                            pnum[:, 128 * c:128 * c + 128],
                            vt[:, vcol],
                            pts[:, 128 * pos:128 * pos + 128],
                            start=first, stop=last,
                            skip_group_check=not (first or last))
                return emit

            def make_norm(h, pnum):
                pr, half = h // 2, h % 2
                r0 = 64 * half

                def emit():
                    # denominator row out of PSUM, fast reciprocal, broadcast,
                    # normalize + fold the v-bias (probabilities sum to 1)
                    den = denp.tile([1, T], F32, tag="den", name="den")
                    inv = denp.tile([1, T], F32, tag="inv", name="inv")
                    nc.vector.tensor_copy(den[:], pnum[64:65, :])
                    nc.vector.reciprocal_approx_fast(inv[:], den[:])
                    invb = invp.tile([HD, T], F32, tag="invb", name="invb")
                    nc.gpsimd.partition_broadcast(invb[:], inv[:])
                    nc.vector.tensor_tensor(
                        ctxT[pr][r0:r0 + HD, :], pnum[:HD, :], invb[:],
                        ALU.mult)
                    nc.vector.tensor_scalar_add(
                        ctxT[pr][r0:r0 + HD, :], ctxT[pr][r0:r0 + HD, :],
                        bcols[r0:r0 + HD, h:h + 1])
                return emit

            for h in range(H):
                pr, half = h // 2, h % 2
                r0 = 64 * half
                pnum = pnp.tile([65, T], F32, tag="num", name="num")
                for c in range(NCH):
                    sc = scp.tile([128, 768], F32, tag="sc", name="sc")
                    qs = qT[pr][r0:r0 + 64, 128 * c:128 * c + 128]
                    # one accumulation group per 2KB PSUM bank: first matmul
                    # of the bank starts (whole bank goes pending-zero, so
                    # later block writes overwrite), last matmul stops.
                    bank_mms = {0: [], 1: []}
                    for pos in range(5):
                        t = c + POS2I[pos]
                        bank_mms[pos // 4].append((
                            sc[:, 128 * pos:128 * pos + 128],
                            kT[pr][r0:r0 + 64, G + 128 * t:G + 128 * t + 128],
                            qs))
                    bank_mms[1].append((
                        sc[:, 640:768], kT[pr][r0:r0 + 64, 0:G], qs))
                    for (p0, nb), mboff in zip(MASK_SPANS[c], MB_OFF[c]):
                        # full-row mask matmul: overlaps both row groups, so
                        # the PE serializes it against the 64-row score
                        # matmuls (cross-row-group concurrent writes to one
                        # psum bank hang the device).
                        ncols = 128 * nb
                        bank_mms[p0 // 4].append((
                            sc[:, 128 * p0:128 * p0 + ncols],
                            diag[:, :], maskb[:, mboff:mboff + ncols]))
                    for bank in (0, 1):
                        mms = bank_mms[bank]
                        for n, (out, lhsT, rhs) in enumerate(mms):
                            first = n == 0
                            last = n == len(mms) - 1
                            nc.tensor.matmul(
                                out, lhsT, rhs, start=first, stop=last,
                                skip_group_check=not (first or last))
                    pts = ptsp.tile([128, 768], BF16, tag="pts", name="pts")
                    nc.scalar.activation(pts[:], sc[:], AF.Exp)
                    # software pipeline: previous chunk's PV (and a finished
                    # head's normalize chain) go behind this chunk's scores
                    if _os.environ.get("NOPIPE") == "1":
                        deferred.append(make_pv(h, c, pts, pnum))
                        flush()
                    else:
                        flush()
                        deferred.append(make_pv(h, c, pts, pnum))
                deferred.append(make_norm(h, pnum))
            flush()

            if _os.environ.get("OG_OFF") != "1":
                # ---- fold the AllReduced global-attention output into ctxT
                # reciprocal_approx_fast mis-reads inputs whose partition
                # base is not 0 (HW ucode quirk) — stage the den row first.
                ogden = dscp.tile([1, H * G], F32, tag="ogden", name="ogden",
                                  bufs=1)
                nc.vector.tensor_copy(ogden[:], ogred[64:65, :])
                ogi = dscp.tile([1, H * G], F32, tag="ogi", name="ogi", bufs=1)
                nc.vector.reciprocal_approx_fast(ogi[:], ogden[:])
                oginvb = dscp.tile([HD, H * G], F32, tag="oginvb",
                                   name="oginvb", bufs=1)
                nc.gpsimd.partition_broadcast(oginvb[:], ogi[:])
                for h in range(H):
                    pr, half = h // 2, h % 2
                    r0 = 64 * half
                    ogt = dscp.tile([128, G], BF16, tag="ogt", name="ogt",
                                    bufs=2)
                    nc.vector.tensor_tensor(
                        ogt[r0:r0 + HD, :], ogred[:HD, G * h:G * h + G],
                        oginvb[:, G * h:G * h + G], ALU.mult)
                    nc.vector.tensor_scalar_add(
                        ogt[r0:r0 + HD, :], ogt[r0:r0 + HD, :],
                        bcols[r0:r0 + HD, 12 + h:13 + h])
                    nc.vector.tensor_scalar_mul(
                        ctxT[pr][r0:r0 + HD, :G], ctxT[pr][r0:r0 + HD, :G],
                        msel[r0:r0 + HD, 1:2])
                    nc.vector.scalar_tensor_tensor(
                        ctxT[pr][r0:r0 + HD, :G], ogt[r0:r0 + HD, :],
                        msel[r0:r0 + HD, 0:1],
                        ctxT[pr][r0:r0 + HD, :G], ALU.mult, ALU.add)

        # ---- output Dense + residual + LayerNorm ----
        with (
            tc.tile_pool(name="wo", bufs=1) as wop,
            tc.tile_pool(name="ln", bufs=3) as lnp,
            tc.tile_pool(name="psd", bufs=4, space="PSUM") as psd,
        ):
            wo = []
            for k in range(6):
                t_ = wop.tile([128, D], BF16, tag=f"wo{k}", name=f"wo{k}")
                nc.sync.dma_start(t_[:], w_d[6, 128 * k:128 * k + 128, :])
                wo.append(t_)
            epst = wop.tile([128, 1], F32, tag="epst", name="epst")
            nc.gpsimd.memset(epst[:], EPS)
            for m in list(range(1, 8)) + [0]:
                ys = lnp.tile([128, D], F32, tag="ys", name="ys")
                rs = lnp.tile([128, D], F32, tag="rs", name="rs")
                nc.sync.dma_start(rs[:], res_d[128 * m:128 * m + 128, :])
                sums = lnp.tile([128, 4], F32, tag="sums", name="sums")
                for n0, nw in ((0, 512), (512, 256)):
                    ps = psd.tile([128, 512], F32, tag="sc", name="sc")
                    for k in range(6):
                        nc.tensor.matmul(
                            ps[:, :nw],
                            ctxT[k][:, 128 * m:128 * m + 128],
                            wo[k][:, n0:n0 + nw],
                            start=(k == 0), stop=(k == 5))
                    nc.vector.scalar_tensor_tensor(
                        ys[:, n0:n0 + nw], ps[:, :nw], 1.0, rs[:, n0:n0 + nw],
                        ALU.mult, ALU.add,
                        accum_out=sums[:, (0 if n0 == 0 else 1):(1 if n0 == 0 else 2)])
                nc.vector.tensor_add(sums[:, 2:3], sums[:, 0:1], sums[:, 1:2])
                negmean = lnp.tile([128, 1], F32, tag="negmean", name="negmean")
                nc.vector.tensor_scalar_mul(negmean[:], sums[:, 2:3], -1.0 / D)
                yc = lnp.tile([128, D], F32, tag="yc", name="yc")
                nc.vector.tensor_scalar_add(yc[:], ys[:], negmean[:])
                sq = lnp.tile([128, D], F32, tag="sq", name="sq")
                sumsq = lnp.tile([128, 1], F32, tag="sumsq", name="sumsq")
                nc.scalar.activation(sq[:], yc[:], AF.Square,
                                     accum_out=sumsq[:])
                sd = lnp.tile([128, 1], F32, tag="sd", name="sd")
                nc.scalar.activation(sd[:], sumsq[:], AF.Sqrt,
                                     bias=epst[:], scale=1.0 / D)
                istd = lnp.tile([128, 1], F32, tag="istd", name="istd")
                nc.vector.reciprocal_approx_fast(istd[:], sd[:])
                yo = lnp.tile([128, D], F32, tag="yo", name="yo")
                nc.vector.scalar_tensor_tensor(
                    yo[:], yc[:], istd[:], vrow["gam"][:], ALU.mult, ALU.mult)
                nc.vector.tensor_tensor(
                    yo[:], yo[:], vrow["bet"][:], ALU.add)
                nc.sync.dma_start(y_d[128 * m:128 * m + 128, :], yo[:])


def build_nc():
    nc = bacc.Bacc(trn_type="TRN2", num_devices=8)
    dt = {
        "xT": nc.dram_tensor("xT", [D, KT], BF16, kind="ExternalInput"),
        "w": nc.dram_tensor("w", [7, D, D], BF16, kind="ExternalInput"),
        "res": nc.dram_tensor("res", [T, D], F32, kind="ExternalInput"),
        "maskb": nc.dram_tensor("maskb", [128, MASK_COLS], BF16,
                                kind="ExternalInput"),
        "diag": nc.dram_tensor("diag", [128, 128], BF16, kind="ExternalInput"),
        "msel": nc.dram_tensor("msel", [128, 2], F32, kind="ExternalInput"),
        "vrep": nc.dram_tensor("vrep", [2, 128, D], F32, kind="ExternalInput"),
        "biasT": nc.dram_tensor("biasT", [128, 24], F32, kind="ExternalInput"),
        "bcols": nc.dram_tensor("bcols", [128, 24], F32, kind="ExternalInput"),
        "y": nc.dram_tensor("y", [T, D], F32, kind="ExternalOutput"),
    }
    with tile.TileContext(nc) as tc:
        _emit(tc, dt)
    nc.compile()
    return nc


def host_inputs(inputs):
    """Build the 8 per-core input maps from the full problem inputs."""
    hs = np.asarray(inputs["hidden_states"], np.float32)
    assert hs.shape == (B, S, D)
    bf = lambda a: np.ascontiguousarray(np.asarray(a, np.float32)).astype(
        ml_dtypes.bfloat16)
    f32 = lambda a: np.ascontiguousarray(np.asarray(a, np.float32))

    wstack = np.stack([
        np.asarray(inputs["Wq"], np.float32) * SCALE,
        np.asarray(inputs["Wk"], np.float32),
        np.asarray(inputs["Wv"], np.float32),
        np.asarray(inputs["Wkg"], np.float32),
        np.asarray(inputs["Wvg"], np.float32),
        np.asarray(inputs["Wqg"], np.float32) * SCALE,
        np.asarray(inputs["Wo"], np.float32),
    ])
    bq = np.asarray(inputs["bq"], np.float32) * SCALE
    bqg = np.asarray(inputs["bqg"], np.float32) * SCALE
    bv = np.asarray(inputs["bv"], np.float32)
    bvg = np.asarray(inputs["bvg"], np.float32)
    biasT = np.zeros((128, 24), np.float32)
    for p in range(6):
        biasT[:, 0 * 6 + p] = bq[128 * p:128 * p + 128]
        biasT[:, 3 * 6 + p] = bqg[128 * p:128 * p + 128]
    # bcols[p, h]   = bv[64h + p%64]   (v-bias fold after normalize)
    # bcols[p, 12+h]= bvg[64h + p%64]
    bcolsv = np.zeros((128, 24), np.float32)
    pm = np.arange(128) % 64
    for h in range(H):
        bcolsv[:, h] = bv[64 * h + pm]
        bcolsv[:, 12 + h] = bvg[64 * h + pm]
    vrep = np.ascontiguousarray(np.broadcast_to(
        np.stack([
            np.asarray(inputs["ln_gamma"], np.float32),
            np.asarray(inputs["ln_beta"], np.float32),
        ])[:, None, :], (2, 128, D)))
    bo = np.asarray(inputs["bo"], np.float32)
    diag = (MASK_NEG * np.eye(128, dtype=np.float32))

    w_bf = bf(wstack)
    diag_bf = bf(diag)

    in_maps = []
    for c in range(8):
        b, j = c // 4, c % 4
        r0 = j * T
        x = hs[b]
        xp = np.zeros((S + 2 * W, D), np.float32)
        xp[W:W + S] = x
        x_kv = np.concatenate([x[:G], xp[r0:r0 + HALO]], axis=0)  # [1664, D]
        xT = bf(x_kv.T)
        res = f32(x[r0:r0 + T] + bo)

        mask = np.zeros((128, MASK_COLS), np.float32)
        jj = np.arange(128)[:, None]       # kcol within tile
        qi = np.arange(128)[None, :]       # query within chunk
        for ch in range(NCH):
            for (p0, nb), off in zip(MASK_SPANS[ch], MB_OFF[ch]):
                for bb in range(nb):
                    i = POS2I[p0 + bb]
                    t = ch + i
                    kpos = r0 - W + 128 * t + jj
                    rel = 128 * i - W + jj - qi
                    valid = ((np.abs(rel) <= W) & (kpos >= G) & (kpos < S))
                    mask[:, off + 128 * bb:off + 128 * bb + 128] = (
                        (~valid).astype(np.float32))

        m = 1.0 if j == 0 else 0.0
        msel = np.zeros((128, 2), np.float32)
        msel[:, 0] = m
        msel[:, 1] = 1.0 - m

        in_maps.append({
            "xT": xT, "w": w_bf, "res": res,
            "maskb": bf(mask), "diag": diag_bf, "msel": f32(msel),
            "vrep": vrep, "biasT": biasT, "bcols": bcolsv,
        })
    return in_maps


_NC_CACHE = {}


def _get_nc():
    if "nc" not in _NC_CACHE:
        _NC_CACHE["nc"] = build_nc()
    return _NC_CACHE["nc"]


def kernel(**inputs) -> np.ndarray:
    # sanity-check the fixed global-attention pattern this kernel hardcodes
    iga = np.asarray(inputs["is_index_global_attn"])
    assert iga.shape == (B, S)
    expect = np.broadcast_to(np.arange(S) < G, (B, S))
    assert np.array_equal(iga, expect), "kernel hardcodes a G=128 prefix"
    am = np.asarray(inputs["attention_mask"], np.float32)
    assert np.all(am == 0.0), "kernel assumes no key-padding mask"

    nc = _get_nc()
    in_maps = host_inputs(inputs)
    res = bass_utils.run_bass_kernel_spmd(nc, in_maps, core_ids=list(range(8)))
    outs = res.results if hasattr(res, "results") else res
    y = np.zeros((B, S, D), np.float32)
    for c in range(8):
        b, j = c // 4, c % 4
        y[b, j * T:(j + 1) * T] = outs[c]["y"]
    return y


if __name__ == "__main__":
    nc = build_nc()
    print("build ok; instructions:",
          sum(len(bb.instructions) for bb in nc.main_func.blocks))


# revision 16
# speedup vs baseline: 1.4244x; 1.0568x over previous
# Trainium2 Bass kernel for nn_LongformerSelfAttentionPegasus (B=2,S=4096,D=768,
# H=12,HD=64, window W=256 one-sided, G=128 global prefix tokens).
#
# Sharding (8 NeuronCores): sequence-parallel — core c handles batch c//4,
# query rows [1024*(c%4), 1024*(c%4+1)). Banded attention is fully local (the
# host ships a +/-W halo of the hidden states). The global-query attention
# (rows 0..G attend to all S tokens through the *_global projections) is
# token-parallel: each core computes exp-score partials (numerator+denominator
# via a ones-column on V) over its own 1024 tokens, and a [65*12, 128]
# AllReduce within each 4-core batch group completes the softmax. The final
# Dense + residual + LayerNorm are row-local, so no further communication.
#
# v3 structure:
#  - phase order keeps every engine fed: kgf/vgf/qg projections, the
#    global-query score matmuls, then the k projection (PE busy while ScalarE
#    drains the ptog exps), og numerators + AllReduce kick, v/q projections,
#    then the band. The PE never waits on a long ScalarE backlog, so the HAM
#    clock gate stays at full rate.
#  - band scores per (head, chunk) into a [128, 768] PSUM tile (5 banded key
#    blocks + 1 global block); ONE exp per chunk writes half of a
#    2-chunk [128, 1536] bf16 pts tile.
#  - band triangle masking is a 0/1 bf16 DVE multiply on the pts tile (two
#    [128,256] multiplies per chunk pair); only 3 small sequence-boundary
#    -30 matmuls per head remain on the PE.
#  - PV processes chunk pairs: the 4 shared key tiles and the global block
#    stream 256 columns per weight load (7 matmuls/pair instead of 12).
#  - softmax reciprocals via vector.reciprocal_approx_fast (input partition
#    base must be 0 — the ucode mis-reads other bases); LayerNorm via
#    Sqrt+reciprocal. Only ~2 ACT table loads total.
#  - PV/normalize work for chunk-pair p is emitted after the scores of pair
#    p+1 so the in-order PE queue never stalls on the exp latency.
#  - v/v_global biases are folded in after the softmax normalize
#    (out = raw/den + bias since probabilities sum to 1); k/k_global biases
#    are dropped entirely (softmax shift invariance).
#
# Hardware gotchas baked in (found the hard way):
#  - matmuls with different row groups (64-row lhsT at partition 0 vs 64)
#    must not write the same PSUM bank while potentially concurrent.
#  - one PSUM accumulation group per 2KB bank: first matmul start=True
#    (whole bank goes pending-zero -> later block writes overwrite), last
#    stop=True, middles skip_group_check.
import sys
import os as _os

for _p in ("/opt/trn_rl_repo",):
    if _p not in sys.path:
        sys.path.insert(0, _p)

import numpy as np
import ml_dtypes

import concourse.bass as bass
import concourse.bacc as bacc
import concourse.mybir as mybir
import concourse.tile as tile
from concourse import bass_utils

F32 = mybir.dt.float32
BF16 = mybir.dt.bfloat16
AF = mybir.ActivationFunctionType
ALU = mybir.AluOpType

B, S, D, H, HD = 2, 4096, 768, 12, 64
W, G = 256, 128
EPS = 1e-5
SCALE = 1.0 / np.sqrt(HD)

T = 1024                 # query rows per core
HALO = T + 2 * W         # 1536 banded kv rows per core
KT = G + HALO            # 1664 total kv rows (128 global + halo)
NCH = 8                  # query chunks of 128
NBT = HALO // 128        # 12 band kcol tiles
NKP = KT // 128          # 13 v partition tiles
MASK_NEG = -30.0

# sc-block order within a chunk: positions 0..4 hold band blocks
# i = 0,4,1,2,3 (i = key-tile offset, t = chunk+i); position 5 = global cols.
# i=0/i=4 lead so the two triangle-masked blocks of a chunk are adjacent
# (one [128,256] DVE mask multiply per chunk).
POS2I = (0, 4, 1, 2, 3)
IPOS = {i: p for p, i in enumerate(POS2I)}
# sequence-boundary -30 matmul spans (pos0, nblocks): chunks 0/1 hit the
# kpos<G boundary on their i=1/2 blocks (cores with r0=0), chunk 7 hits
# kpos>=S on i=3 (cores with r0=3072). i=0/i=4 boundary effects fold into
# the triangle DVE mask data.
BOUND_SPANS = {0: [(2, 2)], 1: [(2, 1)], 7: [(4, 1)]}
_off = 0
BND_OFF = {}
for _c, _sp in BOUND_SPANS.items():
    BND_OFF[_c] = []
    for (_p0, _nb) in _sp:
        BND_OFF[_c].append(_off)
        _off += 128 * _nb
BND_COLS = _off  # 512
TRI_COLS = 4 * 512  # per pair: [chunk-a pos0/1 | chunk-b pos0/1] = 512


def _emit(tc, dt):
    nc = tc.nc
    xT_d, w_d, res_d, maskb_d, diag_d, msel_d, y_d = (
        dt["xT"], dt["w"], dt["res"], dt["maskb"], dt["diag"],
        dt["msel"], dt["y"])
    vrep_d = dt["vrep"]
    og_on = _os.environ.get("OG_OFF") != "1"

    with (
        tc.tile_pool(name="const", bufs=1) as constp,
        tc.tile_pool(name="persist", bufs=1) as pers,
    ):
        # ---- ACT table warmup: exp set only (identity/square/copy share it)
        warm = constp.tile([1, 16], F32, tag="warm", name="warm")
        nc.vector.memset(warm[:], 1.0)
        nc.scalar.activation(warm[:], warm[:], AF.Exp)
        nc.scalar.activation(warm[:], warm[:], AF.Square)
        nc.scalar.activation(warm[:], warm[:], AF.Identity, bias=warm[:, 0:1])

        # ---- constants ----
        diag = constp.tile([128, 128], BF16, tag="diag", name="diag")
        nc.sync.dma_start(diag[:], diag_d[:])
        maskb = constp.tile([128, BND_COLS], BF16, tag="maskb", name="maskb")
        nc.sync.dma_start(maskb[:], maskb_d[:])
        trimask = constp.tile([128, TRI_COLS], BF16, tag="trimask",
                              name="trimask")
        nc.sync.dma_start(trimask[:], dt["trimask"][:])
        msel = constp.tile([128, 2], F32, tag="msel", name="msel")
        nc.sync.dma_start(msel[:], msel_d[:])
        biasT = constp.tile([128, 24], F32, tag="biasT", name="biasT")
        nc.sync.dma_start(biasT[:], dt["biasT"][:])
        bias_t = {}
        for row, name in ((0, "bq"), (3, "bqg")):
            bias_t[name] = [biasT[:, row * 6 + p:row * 6 + p + 1]
                            for p in range(6)]
        bcols = constp.tile([128, 24], F32, tag="bcols", name="bcols")
        nc.sync.dma_start(bcols[:], dt["bcols"][:])
        vrow = {}
        for row, name in ((0, "gam"), (1, "bet")):
            t_ = constp.tile([128, D], F32, tag=name)
            nc.sync.dma_start(t_[:], vrep_d[row])
            vrow[name] = t_

        # ---- persistent activation storage (bf16) ----
        kT = [pers.tile([128, KT], BF16, tag=f"kT{p}", name=f"kT{p}") for p in range(6)]
        qT = [pers.tile([128, T], BF16, tag=f"qT{p}", name=f"qT{p}") for p in range(6)]
        kgfT = [pers.tile([128, T], BF16, tag=f"kgfT{p}", name=f"kgfT{p}") for p in range(6)]
        qgT = [pers.tile([128, G], BF16, tag=f"qgT{p}", name=f"qgT{p}") for p in range(6)]
        vsb = [pers.tile([128, H * (HD + 1)], BF16, tag=f"v{p}", name=f"v{p}") for p in range(NKP)]
        vgf = [pers.tile([128, H * (HD + 1)], BF16, tag=f"vg{p}", name=f"vg{p}") for p in range(8)]
        ctxT = [pers.tile([128, T], BF16, tag=f"ctxT{p}", name=f"ctxT{p}") for p in range(6)]
        ogsb = pers.tile([65, H * G], F32, tag="ogsb", name="ogsb")
        ogred = pers.tile([65, H * G], F32, tag="ogred", name="ogred")

        # ones columns of the v tiles (ones-trick denominator), strided memset
        for p in range(NKP):
            ov = vsb[p][:].rearrange("p (h e) -> p h e", e=HD + 1)
            nc.vector.memset(ov[:, :, HD:HD + 1], 1.0)
        for p in range(8):
            ov = vgf[p][:].rearrange("p (h e) -> p h e", e=HD + 1)
            nc.vector.memset(ov[:, :, HD:HD + 1], 1.0)

        # ---- projections + global-query partials, interleaved so the PE
        # ---- keeps running while ScalarE drains the ptog exps ----
        with (
            tc.tile_pool(name="xw", bufs=2) as xw,
            tc.tile_pool(name="psp", bufs=2, space="PSUM") as psp,
            tc.tile_pool(name="pps", bufs=2, space="PSUM") as pps,
            tc.tile_pool(name="ogn", bufs=2, space="PSUM") as ognp,
            tc.tile_pool(name="ptog", bufs=6) as ptogp,
        ):
            xT = [xw.tile([128, KT], BF16, tag=f"xT{p}", name=f"xT{p}")
                  for p in range(6)]
            for p in range(6):
                nc.sync.dma_start(xT[p][:], xT_d[128 * p:128 * p + 128, :])

            def load_w(widx):
                tiles = []
                for k in range(6):
                    t_ = xw.tile([128, D], BF16, tag=f"w{k}", name=f"w{k}")
                    nc.sync.dma_start(t_[:], w_d[widx, 128 * k:128 * k + 128, :])
                    tiles.append(t_)
                return tiles

            def proj_T(wt, out_tiles, bias, xcol0, ncols):
                # out[dout, tok] = W.T @ x.T  (lhsT=W tile, rhs=xT slice)
                for m in range(6):
                    nn = 0
                    while nn < ncols:
                        nw = min(512, ncols - nn)
                        ps = psp.tile([128, 512], F32, tag="sc", name="sc")
                        for k in range(6):
                            nc.tensor.matmul(
                                ps[:, :nw],
                                wt[k][:, 128 * m:128 * m + 128],
                                xT[k][:, xcol0 + nn:xcol0 + nn + nw],
                                start=(k == 0), stop=(k == 5))
                        if bias is None:
                            nc.vector.tensor_copy(
                                out_tiles[m][:, nn:nn + nw], ps[:, :nw])
                        else:
                            nc.vector.tensor_scalar_add(
                                out_tiles[m][:, nn:nn + nw], ps[:, :nw],
                                bias[m][:])
                        nn += nw

            def proj_N(wt, out_tiles, xcol0, ntok):
                # out[tok, dout] natural layout, strided (HD+1) per head.
                for m in range(ntok // 128):
                    for n0, nw in ((0, 512), (512, 256)):
                        ps = psp.tile([128, 512], F32, tag="sc", name="sc")
                        for k in range(6):
                            nc.tensor.matmul(
                                ps[:, :nw],
                                xT[k][:, xcol0 + 128 * m:xcol0 + 128 * m + 128],
                                wt[k][:, n0:n0 + nw],
                                start=(k == 0), stop=(k == 5))
                        ov = out_tiles[m][:].rearrange("p (h e) -> p h e", e=HD + 1)
                        h0 = n0 // HD
                        nh = nw // HD
                        nc.vector.tensor_copy(
                            ov[:, h0:h0 + nh, :HD],
                            ps[:, :nw].rearrange("p (h e) -> p h e", e=HD))

            wkg = load_w(3)
            proj_T(wkg, kgfT, None, G + W, T)
            wvg = load_w(4)
            proj_N(wvg, vgf, G + W, T)
            wqg = load_w(5)
            proj_T(wqg, qgT, bias_t["bqg"], 0, G)

            ptq = {}
            if og_on:
                # global-query score partials: [128, 1024] psum per tt-quad,
                # half 0 -> bank 0 (cols 0:512), half 1 -> bank 1 so the
                # row-group-concurrent matmul pairs drain to different banks.
                for pr in range(6):
                    for q in range(2):
                        ps = pps.tile([128, 1024], F32, tag="pp", name="pp")
                        for k in range(4):
                            tt = 4 * q + k
                            for half in (0, 1):
                                r0 = 64 * half
                                nc.tensor.matmul(
                                    ps[:, 512 * half + 128 * k:
                                       512 * half + 128 * k + 128],
                                    kgfT[pr][r0:r0 + 64,
                                             128 * tt:128 * tt + 128],
                                    qgT[pr][r0:r0 + 64, :],
                                    start=True, stop=True)
                        pt = ptogp.tile([128, 1024], BF16, tag="ptog",
                                        name="ptog")
                        nc.scalar.activation(pt[:], ps[:], AF.Exp)
                        ptq[(pr, q)] = pt

            # k projection: PE stays busy while ScalarE drains the ptog exps
            wk = load_w(1)
            proj_T(wk, kT, None, 0, KT)

            if og_on:
                # og numerators (exps are done by now), then kick AllReduce
                for pr in range(6):
                    for half in (0, 1):
                        h = 2 * pr + half
                        vcol = slice((HD + 1) * h, (HD + 1) * h + HD + 1)
                        ps2 = ognp.tile([65, 128], F32, tag="ogn", name="ogn")
                        for tt in range(8):
                            rhs = ptq[(pr, tt // 4)][
                                :, 512 * half + 128 * (tt % 4):
                                512 * half + 128 * (tt % 4) + 128]
                            nc.tensor.matmul(
                                ps2[:, :], vgf[tt][:, vcol], rhs,
                                start=(tt == 0), stop=(tt == 7))
                        nc.scalar.copy(ogsb[:, G * h:G * h + G], ps2[:])
                with tc.tile_pool(name="ogdram", bufs=1, space="DRAM") as ogd:
                    og_in = ogd.tile([65, H * G], F32, tag="og_in",
                                     name="og_in")
                    og_out = ogd.tile([65, H * G], F32, tag="og_out",
                                      name="og_out")
                    nc.sync.dma_start(og_in[:], ogsb[:])
                    if _os.environ.get("NO_CC") == "1":
                        nc.sync.dma_start(og_out[:], og_in[:])
                    else:
                        nc.gpsimd.collective_compute(
                            "AllReduce", ALU.add,
                            replica_groups=[[0, 1, 2, 3], [4, 5, 6, 7]],
                            ins=[og_in.opt()], outs=[og_out.opt()])
                    nc.sync.dma_start(ogred[:], og_out[:])

            wv = load_w(2)
            proj_N(wv, vsb, 0, KT)
            wq = load_w(0)
            proj_T(wq, qT, bias_t["bq"], G + W, T)

        # ---- banded local attention ----
        with (
            tc.tile_pool(name="scp", bufs=2, space="PSUM") as scp,
            tc.tile_pool(name="pnum", bufs=2, space="PSUM") as pnp,
            tc.tile_pool(name="ptsp", bufs=2) as ptsp,
            tc.tile_pool(name="invp", bufs=3) as invp,
            tc.tile_pool(name="denp", bufs=4) as denp,
            tc.tile_pool(name="dsc", bufs=2) as dscp,
        ):
            if og_on:
                # og softmax completion: reciprocal of the AllReduced
                # denominator row (staged to partition base 0 first), then a
                # broadcast. Emitted up front; runs once the DMA lands.
                ogden = dscp.tile([1, H * G], F32, tag="ogden", name="ogden",
                                  bufs=1)
                nc.vector.tensor_copy(ogden[:], ogred[64:65, :])
                ogi = dscp.tile([1, H * G], F32, tag="ogi", name="ogi",
                                bufs=1)
                nc.vector.reciprocal_approx_fast(ogi[:], ogden[:])
                oginvb = dscp.tile([HD, H * G], F32, tag="oginvb",
                                   name="oginvb", bufs=1)
                nc.gpsimd.partition_broadcast(oginvb[:], ogi[:])

            deferred = []

            def flush():
                while deferred:
                    deferred.pop(0)()

            def make_pv_pair(h, c, pts, pnum):
                # PV for chunk pair (c, c+1): shared key tiles and the global
                # block stream both chunks' probabilities per weight load.
                vcol = slice((HD + 1) * h, (HD + 1) * h + HD + 1)
                ptsv = pts[:].rearrange("p (u x) -> p u x", x=128)

                def emit():
                    mms = []
                    # shared band tiles t=c+1..c+4: chunk c block i=t-c,
                    # chunk c+1 block i'=t-c-1 (stepped slice -> [128,2,128])
                    for t in range(c + 1, c + 5):
                        pa = IPOS[t - c]
                        pb = 6 + IPOS[t - c - 1]
                        mms.append((vsb[1 + t],
                                    ptsv[:, pa:pb + 1:pb - pa, :],
                                    pnum[:, 128 * c:128 * c + 256]))
                    # global block, both chunks
                    mms.append((vsb[0], ptsv[:, 5:12:6, :],
                                pnum[:, 128 * c:128 * c + 256]))
                    # exclusive tiles t=c (chunk c only), t=c+5 (chunk c+1)
                    mms.append((vsb[1 + c], ptsv[:, 0:1, :],
                                pnum[:, 128 * c:128 * c + 128]))
                    pe = 6 + IPOS[4]
                    mms.append((vsb[1 + c + 5], ptsv[:, pe:pe + 1, :],
                                pnum[:, 128 * (c + 1):128 * (c + 1) + 128]))
                    first_of_bank = c in (0, 4)
                    last_of_bank = c in (2, 6)
                    for n, (vt, rhs, out) in enumerate(mms):
                        first = first_of_bank and n == 0
                        last = last_of_bank and n == len(mms) - 1
                        nc.tensor.matmul(
                            out, vt[:, vcol], rhs,
                            start=first, stop=last,
                            skip_group_check=not (first or last))
                return emit

            def make_norm(h, pnum, n0, nw):
                pr, half = h // 2, h % 2
                r0 = 64 * half

                def emit():
                    den = denp.tile([1, T], F32, tag="den", name="den")
                    inv = denp.tile([1, T], F32, tag="inv", name="inv")
                    nc.vector.tensor_copy(den[:, :nw], pnum[64:65, n0:n0 + nw])
                    nc.vector.reciprocal_approx_fast(
                        inv[:, :nw], den[:, :nw])
                    invb = invp.tile([HD, T], F32, tag="invb", name="invb")
                    nc.gpsimd.partition_broadcast(
                        invb[:, :nw], inv[:, :nw], channels=HD)
                    nc.vector.tensor_tensor(
                        ctxT[pr][r0:r0 + HD, n0:n0 + nw],
                        pnum[:HD, n0:n0 + nw], invb[:, :nw], ALU.mult)
                    nc.vector.tensor_scalar_add(
                        ctxT[pr][r0:r0 + HD, n0:n0 + nw],
                        ctxT[pr][r0:r0 + HD, n0:n0 + nw],
                        bcols[r0:r0 + HD, h:h + 1])
                return emit

            def make_ogfold(h):
                pr, half = h // 2, h % 2
                r0 = 64 * half

                def emit():
                    ogt = dscp.tile([128, G], BF16, tag="ogt", name="ogt",
                                    bufs=2)
                    nc.vector.tensor_tensor(
                        ogt[r0:r0 + HD, :], ogred[:HD, G * h:G * h + G],
                        oginvb[:, G * h:G * h + G], ALU.mult)
                    nc.vector.tensor_scalar_add(
                        ogt[r0:r0 + HD, :], ogt[r0:r0 + HD, :],
                        bcols[r0:r0 + HD, 12 + h:13 + h])
                    nc.vector.tensor_scalar_mul(
                        ctxT[pr][r0:r0 + HD, :G], ctxT[pr][r0:r0 + HD, :G],
                        msel[r0:r0 + HD, 1:2])
                    nc.vector.scalar_tensor_tensor(
                        ctxT[pr][r0:r0 + HD, :G], ogt[r0:r0 + HD, :],
                        msel[r0:r0 + HD, 0:1],
                        ctxT[pr][r0:r0 + HD, :G], ALU.mult, ALU.add)
                return emit

            for h in range(H):
                pr, half = h // 2, h % 2
                r0 = 64 * half
                pnum = pnp.tile([65, T], F32, tag="num", name="num")
                for cp in range(4):
                    pts = ptsp.tile([128, 1536], BF16, tag="pts", name="pts")
                    for ci in (0, 1):
                        c = 2 * cp + ci
                        sc = scp.tile([128, 768], F32, tag="sc", name="sc")
                        qs = qT[pr][r0:r0 + 64, 128 * c:128 * c + 128]
                        bank_mms = {0: [], 1: []}
                        for pos in range(5):
                            t = c + POS2I[pos]
                            bank_mms[pos // 4].append((
                                sc[:, 128 * pos:128 * pos + 128],
                                kT[pr][r0:r0 + 64,
                                       G + 128 * t:G + 128 * t + 128],
                                qs))
                        bank_mms[1].append((
                            sc[:, 640:768], kT[pr][r0:r0 + 64, 0:G], qs))
                        for (p0, nb), mboff in zip(
                                BOUND_SPANS.get(c, ()), BND_OFF.get(c, ())):
                            ncols = 128 * nb
                            bank_mms[p0 // 4].append((
                                sc[:, 128 * p0:128 * p0 + ncols],
                                diag[:, :], maskb[:, mboff:mboff + ncols]))
                        for bank in (0, 1):
                            mms = bank_mms[bank]
                            for n, (out, lhsT, rhs) in enumerate(mms):
                                first = n == 0
                                last = n == len(mms) - 1
                                nc.tensor.matmul(
                                    out, lhsT, rhs, start=first, stop=last,
                                    skip_group_check=not (first or last))
                        nc.scalar.activation(
                            pts[:, 768 * ci:768 * ci + 768], sc[:], AF.Exp)
                        # triangle (+edge) mask on the i=0 / i=4 blocks
                        nc.vector.tensor_tensor(
                            pts[:, 768 * ci:768 * ci + 256],
                            pts[:, 768 * ci:768 * ci + 256],
                            trimask[:, 512 * cp + 256 * ci:
                                    512 * cp + 256 * ci + 256],
                            ALU.mult)
                    flush()
                    deferred.append(make_pv_pair(h, 2 * cp, pts, pnum))
                    if h == H - 1 and cp == 1:
                        deferred.append(make_norm(h, pnum, 0, 512))
                if h == H - 1:
                    deferred.append(make_norm(h, pnum, 512, 512))
                else:
                    deferred.append(make_norm(h, pnum, 0, T))
                if og_on:
                    deferred.append(make_ogfold(h))
            flush()

        # ---- output Dense + residual + LayerNorm ----
        with (
            tc.tile_pool(name="wo", bufs=1) as wop,
            tc.tile_pool(name="ln", bufs=3) as lnp,
            tc.tile_pool(name="psd", bufs=4, space="PSUM") as psd,
        ):
            wo = []
            for k in range(6):
                t_ = wop.tile([128, D], BF16, tag=f"wo{k}", name=f"wo{k}")
                nc.sync.dma_start(t_[:], w_d[6, 128 * k:128 * k + 128, :])
                wo.append(t_)
            epst = wop.tile([128, 1], F32, tag="epst", name="epst")
            nc.gpsimd.memset(epst[:], EPS)
            for m in list(range(1, 8)) + [0]:
                ys = lnp.tile([128, D], F32, tag="ys", name="ys")
                rs = lnp.tile([128, D], F32, tag="rs", name="rs")
                nc.sync.dma_start(rs[:], res_d[128 * m:128 * m + 128, :])
                sums = lnp.tile([128, 4], F32, tag="sums", name="sums")
                for n0, nw in ((0, 512), (512, 256)):
                    ps = psd.tile([128, 512], F32, tag="sc", name="sc")
                    for k in range(6):
                        nc.tensor.matmul(
                            ps[:, :nw],
                            ctxT[k][:, 128 * m:128 * m + 128],
                            wo[k][:, n0:n0 + nw],
                            start=(k == 0), stop=(k == 5))
                    nc.vector.scalar_tensor_tensor(
                        ys[:, n0:n0 + nw], ps[:, :nw], 1.0, rs[:, n0:n0 + nw],
                        ALU.mult, ALU.add,
                        accum_out=sums[:, (0 if n0 == 0 else 1):(1 if n0 == 0 else 2)])
                nc.vector.tensor_add(sums[:, 2:3], sums[:, 0:1], sums[:, 1:2])
                negmean = lnp.tile([128, 1], F32, tag="negmean", name="negmean")
                nc.vector.tensor_scalar_mul(negmean[:], sums[:, 2:3], -1.0 / D)
                yc = lnp.tile([128, D], F32, tag="yc", name="yc")
                nc.vector.tensor_scalar_add(yc[:], ys[:], negmean[:])
                sq = lnp.tile([128, D], F32, tag="sq", name="sq")
                sumsq = lnp.tile([128, 1], F32, tag="sumsq", name="sumsq")
                nc.scalar.activation(sq[:], yc[:], AF.Square,
                                     accum_out=sumsq[:])
                sd = lnp.tile([128, 1], F32, tag="sd", name="sd")
                nc.scalar.activation(sd[:], sumsq[:], AF.Sqrt,
                                     bias=epst[:], scale=1.0 / D)
                istd = lnp.tile([128, 1], F32, tag="istd", name="istd")
                nc.vector.reciprocal_approx_fast(istd[:], sd[:])
                yo = lnp.tile([128, D], F32, tag="yo", name="yo")
                nc.vector.scalar_tensor_tensor(
                    yo[:], yc[:], istd[:], vrow["gam"][:], ALU.mult, ALU.mult)
                nc.vector.tensor_tensor(
                    yo[:], yo[:], vrow["bet"][:], ALU.add)
                nc.sync.dma_start(y_d[128 * m:128 * m + 128, :], yo[:])


def build_nc():
    nc = bacc.Bacc(trn_type="TRN2", num_devices=8)
    dt = {
        "xT": nc.dram_tensor("xT", [D, KT], BF16, kind="ExternalInput"),
        "w": nc.dram_tensor("w", [7, D, D], BF16, kind="ExternalInput"),
        "res": nc.dram_tensor("res", [T, D], F32, kind="ExternalInput"),
        "maskb": nc.dram_tensor("maskb", [128, BND_COLS], BF16,
                                kind="ExternalInput"),
        "trimask": nc.dram_tensor("trimask", [128, TRI_COLS], BF16,
                                  kind="ExternalInput"),
        "diag": nc.dram_tensor("diag", [128, 128], BF16, kind="ExternalInput"),
        "msel": nc.dram_tensor("msel", [128, 2], F32, kind="ExternalInput"),
        "vrep": nc.dram_tensor("vrep", [2, 128, D], F32, kind="ExternalInput"),
        "biasT": nc.dram_tensor("biasT", [128, 24], F32, kind="ExternalInput"),
        "bcols": nc.dram_tensor("bcols", [128, 24], F32, kind="ExternalInput"),
        "y": nc.dram_tensor("y", [T, D], F32, kind="ExternalOutput"),
    }
    with tile.TileContext(nc) as tc:
        _emit(tc, dt)
    nc.compile()
    return nc


def host_inputs(inputs):
    """Build the 8 per-core input maps from the full problem inputs."""
    hs = np.asarray(inputs["hidden_states"], np.float32)
    assert hs.shape == (B, S, D)
    bf = lambda a: np.ascontiguousarray(np.asarray(a, np.float32)).astype(
        ml_dtypes.bfloat16)
    f32 = lambda a: np.ascontiguousarray(np.asarray(a, np.float32))

    wstack = np.stack([
        np.asarray(inputs["Wq"], np.float32) * SCALE,
        np.asarray(inputs["Wk"], np.float32),
        np.asarray(inputs["Wv"], np.float32),
        np.asarray(inputs["Wkg"], np.float32),
        np.asarray(inputs["Wvg"], np.float32),
        np.asarray(inputs["Wqg"], np.float32) * SCALE,
        np.asarray(inputs["Wo"], np.float32),
    ])
    bq = np.asarray(inputs["bq"], np.float32) * SCALE
    bqg = np.asarray(inputs["bqg"], np.float32) * SCALE
    bv = np.asarray(inputs["bv"], np.float32)
    bvg = np.asarray(inputs["bvg"], np.float32)
    biasT = np.zeros((128, 24), np.float32)
    for p in range(6):
        biasT[:, 0 * 6 + p] = bq[128 * p:128 * p + 128]
        biasT[:, 3 * 6 + p] = bqg[128 * p:128 * p + 128]
    bcolsv = np.zeros((128, 24), np.float32)
    pm = np.arange(128) % 64
    for h in range(H):
        bcolsv[:, h] = bv[64 * h + pm]
        bcolsv[:, 12 + h] = bvg[64 * h + pm]
    vrep = np.ascontiguousarray(np.broadcast_to(
        np.stack([
            np.asarray(inputs["ln_gamma"], np.float32),
            np.asarray(inputs["ln_beta"], np.float32),
        ])[:, None, :], (2, 128, D)))
    bo = np.asarray(inputs["bo"], np.float32)
    diag = (MASK_NEG * np.eye(128, dtype=np.float32))

    w_bf = bf(wstack)
    diag_bf = bf(diag)

    def block_valid(r0, ch, i):
        jj = np.arange(128)[:, None]       # kcol within tile
        qi = np.arange(128)[None, :]       # query within chunk
        t = ch + i
        kpos = r0 - W + 128 * t + jj
        rel = 128 * i - W + jj - qi
        return ((np.abs(rel) <= W) & (kpos >= G) & (kpos < S))

    in_maps = []
    for c in range(8):
        b, j = c // 4, c % 4
        r0 = j * T
        x = hs[b]
        xp = np.zeros((S + 2 * W, D), np.float32)
        xp[W:W + S] = x
        x_kv = np.concatenate([x[:G], xp[r0:r0 + HALO]], axis=0)  # [1664, D]
        xT = bf(x_kv.T)
        res = f32(x[r0:r0 + T] + bo)

        # sequence-boundary -30 mask blocks (1.0 = invalid)
        mask = np.zeros((128, BND_COLS), np.float32)
        for ch, spans in BOUND_SPANS.items():
            for (p0, nb), off in zip(spans, BND_OFF[ch]):
                for bb in range(nb):
                    i = POS2I[p0 + bb]
                    mask[:, off + 128 * bb:off + 128 * bb + 128] = (
                        (~block_valid(r0, ch, i)).astype(np.float32))

        # triangle (+edge) 0/1 multiply masks for the i=0 / i=4 blocks:
        # per chunk pair cp: [chunk 2cp pos0|pos1 , chunk 2cp+1 pos0|pos1]
        tri = np.zeros((128, TRI_COLS), np.float32)
        for cp in range(4):
            for ci in (0, 1):
                ch = 2 * cp + ci
                for bb, i in enumerate((POS2I[0], POS2I[1])):
                    off = 512 * cp + 256 * ci + 128 * bb
                    tri[:, off:off + 128] = block_valid(r0, ch, i).astype(
                        np.float32)

        m = 1.0 if j == 0 else 0.0
        msel = np.zeros((128, 2), np.float32)
        msel[:, 0] = m
        msel[:, 1] = 1.0 - m

        in_maps.append({
            "xT": xT, "w": w_bf, "res": res,
            "maskb": bf(mask), "trimask": bf(tri), "diag": diag_bf,
            "msel": f32(msel), "vrep": vrep, "biasT": biasT, "bcols": bcolsv,
        })
    return in_maps


_NC_CACHE = {}


def _get_nc():
    if "nc" not in _NC_CACHE:
        _NC_CACHE["nc"] = build_nc()
    return _NC_CACHE["nc"]


def kernel(**inputs) -> np.ndarray:
    # sanity-check the fixed global-attention pattern this kernel hardcodes
    iga = np.asarray(inputs["is_index_global_attn"])
    assert iga.shape == (B, S)
    expect = np.broadcast_to(np.arange(S) < G, (B, S))
    assert np.array_equal(iga, expect), "kernel hardcodes a G=128 prefix"
    am = np.asarray(inputs["attention_mask"], np.float32)
    assert np.all(am == 0.0), "kernel assumes no key-padding mask"

    nc = _get_nc()
    in_maps = host_inputs(inputs)
    res = bass_utils.run_bass_kernel_spmd(nc, in_maps, core_ids=list(range(8)))
    outs = res.results if hasattr(res, "results") else res
    y = np.zeros((B, S, D), np.float32)
    for c in range(8):
        b, j = c // 4, c % 4
        y[b, j * T:(j + 1) * T] = outs[c]["y"]
    return y


if __name__ == "__main__":
    nc = build_nc()
    print("build ok; instructions:",
          sum(len(bb.instructions) for bb in nc.main_func.blocks))
